# revision 67
# baseline (speedup 1.0000x reference)
"""
BasicCrossAttention Trainium2 kernel (8 NeuronCores, SPMD head-parallel).

Sharding: 16 heads split across 8 cores (2 heads/core).  Each core computes
Q/K/V projections for its 2 heads (column-sharded Wq/Wk/Wv), per-head QK
LayerNorm, full cross-attention over B*H_local, and a row-sharded partial of
the output projection.  The host sums the 8 fp16 partial outputs + bias.

Key design points (vs the original v1 baseline, 396us -> 324us):
  - x1/x2 are transposed on the HOST; the device loads xT with plain
    contiguous 2KB-descriptor DMA (v1 burned ~115us of DMA-pool time on
    xbar-transpose descriptors).
  - K/Q weight head-blocks are mean-centered on the host (folds the LN
    mean into the weights) and all weights arrive pre-packed in SBUF
    layout; LN needs only E[x^2] on device.
  - Softmax normalizer: Z rows staged to a partition-0 tile, then
    reciprocal_approx_accurate (v1 spent 52us in single-partition
    nc.vector.reciprocal; the custom-DVE op needs partition-aligned
    operands on real HW).
  - rsqrt Newton iteration stays on DVE: putting it on GpSimd creates an
    in-order-queue convoy behind the slow hoT-normalize multiplies that
    stalls PE -> ACT at every chunk boundary.
  - Projection matmul groups yield at 4-matmul granularity so the
    interleave never floods the in-order PE queue between score groups.
  - Output partials stored as fp16 (halves store traffic; host sums fp32).
  - Coarse phase schedule (measured better than every finer interleave
    tried): prod(0); attn(0) || prod(1) 1:1; attn(1) || outp(0) 2:1;
    outp(1).  fp8 DoubleRow AV was tried and works on HW (with 16B-aligned
    padded V slots) but costs too much accuracy (rel err 0.028 > 0.02).
"""

import os
import sys

for _p in ("/root/.axon_site", "/root/.axon_site/_ro/trn_rl_repo",
           "/root/.axon_site/_ro/pypackages", "/opt/trn_rl_repo"):
    if os.path.isdir(_p) and _p not in sys.path:
        sys.path.append(_p)

import numpy as np
import ml_dtypes
from contextlib import ExitStack

B = 2
N = 2048          # query rows (x1)
M = 2048          # key rows (x2)
DM = 1024         # d_model
H = 16            # total heads
HD = 64           # head dim
NCORES = 8
HL = H // NCORES  # heads per core = 2
LOC = HL * HD     # local feature width = 128
SCALE = 8.0 / HD  # mup scale
EPS = 1e-5

_COMPILED = None          # cached Bass program
LAST_RESULT = None        # BassKernelResults of last run (for profiling)


def _emit(ctx, tc, aps):
    import concourse.bass as bass
    from concourse import mybir
    from concourse.masks import make_identity

    nc = tc.nc
    f32 = mybir.dt.float32
    bf16 = mybir.dt.bfloat16
    fp16 = mybir.dt.float16
    AF = mybir.ActivationFunctionType
    OP = mybir.AluOpType

    x1, x2, wkv, wq, wp, gk, bk, gq, bq, out = (
        aps["x1"], aps["x2"], aps["wkv"], aps["wq"],
        aps["wp"], aps["gk"], aps["bk"], aps["gq"], aps["bq"], aps["out"])

    const = ctx.enter_context(tc.tile_pool(name="const", bufs=1))
    xT_pool = ctx.enter_context(tc.tile_pool(name="xTp", bufs=2))
    nat_pool = ctx.enter_context(tc.tile_pool(name="natp", bufs=10))
    stat_pool = ctx.enter_context(tc.tile_pool(name="statp", bufs=3))
    big_pool = ctx.enter_context(tc.tile_pool(name="bigp", bufs=2))
    pT_pool = ctx.enter_context(tc.tile_pool(name="pTp", bufs=3))
    out_pool = ctx.enter_context(tc.tile_pool(name="outp", bufs=3))
    bc_pool = ctx.enter_context(tc.tile_pool(name="bcp", bufs=2))
    ps128 = ctx.enter_context(tc.tile_pool(name="ps128", bufs=2, space="PSUM"))
    psbig = ctx.enter_context(tc.tile_pool(name="psbig", bufs=2, space="PSUM"))

    # ---------------- constants / weights ----------------
    ident = const.tile([128, 128], bf16)
    make_identity(nc, ident)

    # weights arrive host-packed: wkv [128, fc, k|v], wq [128, fc, q]
    wkv_sb = const.tile([128, 8, 2 * LOC], bf16)
    nc.gpsimd.dma_start(wkv_sb, wkv)
    wq_sb = const.tile([128, 8, LOC], bf16)
    nc.gpsimd.dma_start(wq_sb, wq)
    wp_sb = const.tile([128, DM], bf16)
    nc.gpsimd.dma_start(wp_sb, wp)

    g_col = const.tile([128, 1], f32)
    nc.gpsimd.dma_start(g_col, gk)
    b_col = const.tile([128, 1], f32)
    nc.gpsimd.dma_start(b_col, bk)
    gq_col = const.tile([128, 1], f32)
    nc.gpsimd.dma_start(gq_col, gq)
    bq_col = const.tile([128, 1], f32)
    nc.gpsimd.dma_start(bq_col, bq)

    # persistent per-batch tiles (bufs=2 -> both batches in flight)
    kT = [None, None]
    qT = [None, None]
    Vt = [None, None]
    hoT = [None, None]

    # ---------------- phase generators ----------------
    def prod(b):
        """Project K|V (from x2) and Q (from x1) for batch b; LN; transposes."""
        kT[b] = big_pool.tile([128, M], bf16, tag="kT", name=f"kT{b}")
        qT[b] = big_pool.tile([128, N], bf16, tag="qT", name=f"qT{b}")
        Vt[b] = big_pool.tile([128, 16, 2 * (HD + 1)], bf16, tag="V",
                              name=f"V{b}")
        for src, is_q in ((x2, False), (x1, True)):
            w_sb = wq_sb if is_q else wkv_sb
            nout = LOC if is_q else 2 * LOC
            dst = qT[b] if is_q else kT[b]
            gc, bc = (gq_col, bq_col) if is_q else (g_col, b_col)
            for rg in range(2):  # 1024-row groups
                xT = xT_pool.tile([128, 8, 1024], bf16, tag="xT",
                                  name=f"xT{b}{int(is_q)}{rg}")
                for fc in range(8):
                    nc.sync.dma_start(
                        out=xT[:, fc, :],
                        in_=src[b, fc * 128:(fc + 1) * 128,
                                rg * 1024:(rg + 1) * 1024])
                yield
                s2g = stat_pool.tile([128, 8, HL], f32, tag="s2g",
                                     name=f"s2g{b}{int(is_q)}{rg}")
                raws = []
                for mi in range(8):
                    mt = rg * 8 + mi  # global 128-row tile index
                    rs = slice(mi * 128, (mi + 1) * 128)
                    ps = ps128.tile([128, nout], f32, tag="ps128",
                                    name=f"ps{b}{int(is_q)}{mt}")
                    for fc in range(4):
                        nc.tensor.matmul(ps, lhsT=xT[:, fc, rs],
                                         rhs=w_sb[:, fc, :],
                                         start=(fc == 0), stop=False)
                    yield
                    for fc in range(4, 8):
                        nc.tensor.matmul(ps, lhsT=xT[:, fc, rs],
                                         rhs=w_sb[:, fc, :],
                                         start=False, stop=(fc == 7))
                    raw = nat_pool.tile([128, LOC], bf16, tag="raw", bufs=10,
                                        name=f"raw{b}{int(is_q)}{mt}")
                    nc.vector.tensor_copy(raw, ps[:, 0:LOC])
                    raws.append(raw)
                    # E[x^2] per head for LN (weights are centered)
                    sq = nat_pool.tile([128, LOC], f32, tag="sq", bufs=2)
                    nc.vector.tensor_mul(sq, raw, raw)
                    nc.vector.reduce_sum(s2g[:, mi, :],
                                         sq.rearrange("p (h d) -> p h d", h=HL),
                                         axis=mybir.AxisListType.X)
                    if not is_q:
                        vt = Vt[b][:, mt, :]
                        nc.gpsimd.memset(vt[:, HD::HD + 1], 1.0)
                        vt3 = bass.AP(tensor=vt.tensor, offset=vt.offset,
                                      ap=[vt.ap[0], [HD + 1, HL], [1, HD]])
                        nc.vector.tensor_copy(
                            vt3, ps[:, LOC:2 * LOC].rearrange(
                                "p (h x) -> p h x", h=HL))
                    yield
                # per-row-group rstd on DVE: rsqrt(var+eps) via linear seed
                # + 3 Newton steps (keeps ACT exclusively on softmax exp)
                rstdg = stat_pool.tile([128, 8, HL], f32, tag="rstdg")
                y = rstdg.rearrange("p a b -> p (a b)")
                var = stat_pool.tile([128, 8 * HL], f32, tag="lnvar")
                tnr = stat_pool.tile([128, 8 * HL], f32, tag="lntnr")
                nc.vector.tensor_scalar(var, s2g.rearrange("p a b -> p (a b)"),
                                        1.0 / HD, EPS, op0=OP.mult, op1=OP.add)
                nc.vector.tensor_scalar(y, var, -0.315, 1.43,
                                        op0=OP.mult, op1=OP.add)
                for _ in range(3):
                    nc.vector.tensor_mul(tnr, y, y)
                    nc.vector.tensor_mul(tnr, tnr, var)
                    nc.vector.tensor_scalar(tnr, tnr, -0.5, 1.5,
                                            op0=OP.mult, op1=OP.add)
                    nc.vector.tensor_mul(y, y, tnr)
                for mi in range(8):
                    mt = rg * 8 + mi
                    nrm = nat_pool.tile([128, LOC], bf16, tag="nrm", bufs=3)
                    for h in range(HL):
                        hs = slice(h * HD, (h + 1) * HD)
                        nc.vector.tensor_scalar(
                            nrm[:, hs], raws[mi][:, hs],
                            rstdg[:, mi, h:h + 1], None, op0=OP.mult)
                    tps = ps128.tile([128, 128], bf16, tag="ps128",
                                     name=f"tps{b}{int(is_q)}{mt}")
                    nc.tensor.transpose(tps, nrm, ident)
                    nc.vector.tensor_scalar(
                        dst[:, mt * 128:(mt + 1) * 128], tps, gc, bc,
                        op0=OP.mult, op1=OP.add)
                    yield

    def attn(b):
        """S^T -> exp -> (V|1)^T @ P^T, head-pair packed."""
        hoT[b] = big_pool.tile([128, N], bf16, tag="hoT", name=f"hoT{b}")
        for nc4 in range(4):  # 512-wide query column chunks
            ns = slice(nc4 * 512, (nc4 + 1) * 512)
            av = psbig.tile([128, 1024], f32, tag="av", bufs=1,
                            name=f"av{b}{nc4}")
            for mc in range(16):
                mcs = slice(mc * 128, (mc + 1) * 128)
                st = psbig.tile([128, 1024], f32, tag="st",
                                name=f"st{b}{nc4}{mc}")
                for h in range(HL):
                    nc.tensor.matmul(st[:, h * 512:(h + 1) * 512],
                                     lhsT=kT[b][h * HD:(h + 1) * HD, mcs],
                                     rhs=qT[b][h * HD:(h + 1) * HD, ns],
                                     start=True, stop=True)
                pT = pT_pool.tile([128, 1024], bf16, tag="pT")
                nc.scalar.activation(pT, st, AF.Exp)
                for h in range(HL):
                    nc.tensor.matmul(
                        av[0:HD + 1, h * 512:(h + 1) * 512],
                        lhsT=Vt[b][:, mc, h * (HD + 1):(h + 1) * (HD + 1)],
                        rhs=pT[:, h * 512:(h + 1) * 512],
                        start=(mc == 0), stop=(mc == 15),
                        skip_group_check=True)
                yield
            # drain the accumulator to SBUF fast (frees the PSUM bank so the
            # next chunk's AV matmuls never stall on the normalize chain)
            av_sb = bc_pool.tile([HD + 1, 1024], f32, tag="avsb")
            nc.vector.tensor_copy(av_sb, av[0:HD + 1, :])
            zrow = bc_pool.tile([1, 1024], f32, tag="zrow")
            nc.vector.tensor_copy(zrow, av_sb[HD:HD + 1, :])
            recip = bc_pool.tile([1, 1024], f32, tag="recip")
            rscr = bc_pool.tile([1, 1024], f32, tag="rscr")
            nc.vector.reciprocal_approx_accurate(recip, zrow, rscr)
            bcast = bc_pool.tile([HD, 1024], f32, tag="bcast")
            nc.gpsimd.partition_broadcast(bcast, recip)
            for h in range(HL):
                hs = slice(h * HD, (h + 1) * HD)
                sl = slice(h * 512, (h + 1) * 512)
                nc.gpsimd.tensor_mul(hoT[b][hs, ns], av_sb[0:HD, sl],
                                     bcast[:, sl])
                yield

    def outp(b):
        """Output projection partial for batch b."""
        for nt in range(16):
            for oc in range(2):
                fps = psbig.tile([128, 512], f32, tag="st",
                                 name=f"fps{b}{nt}{oc}")
                nc.tensor.matmul(fps,
                                 lhsT=hoT[b][:, nt * 128:(nt + 1) * 128],
                                 rhs=wp_sb[:, oc * 512:(oc + 1) * 512],
                                 start=True, stop=True)
                osb = out_pool.tile([128, 512], fp16, tag="osb")
                if b == 1 and (2 * nt + oc) % 2 == 0:
                    nc.scalar.copy(osb, fps)
                else:
                    nc.vector.tensor_copy(osb, fps)
                nc.sync.dma_start(
                    out[b, nt * 128:(nt + 1) * 128, oc * 512:(oc + 1) * 512],
                    osb)
                yield

    def run_all(g):
        for _ in g:
            pass

    def chain(*gens):
        for g in gens:
            yield from g

    def interleave(ga, gb, ka, kb):
        """Alternate ka steps of ga with kb steps of gb until both drain."""
        alive_a, alive_b = True, True
        while alive_a or alive_b:
            for _ in range(ka):
                if alive_a:
                    alive_a = next(ga, _SENTINEL) is not _SENTINEL
            for _ in range(kb):
                if alive_b:
                    alive_b = next(gb, _SENTINEL) is not _SENTINEL

    def interleave_while_b(ga, gb, ka, kb):
        """Alternate; stop when gb exhausts (ga may have steps left)."""
        while True:
            for _ in range(kb):
                if next(gb, _SENTINEL) is _SENTINEL:
                    return
            for _ in range(ka):
                next(ga, None)

    def interleave_while_a(ga, gb, ka, kb):
        """Alternate; stop when ga exhausts (gb may have steps left)."""
        while True:
            for _ in range(ka):
                if next(ga, _SENTINEL) is _SENTINEL:
                    return
            for _ in range(kb):
                next(gb, None)

    _SENTINEL = object()

    run_all(prod(0))
    interleave(attn(0), prod(1), 1, 1)
    # NOTE: outp(1) must not be emitted before attn(1) finishes the chunks it
    # reads -- Tile dependency tracking is emission-ordered, so an
    # early-emitted reader would race the hoT writes.
    interleave(attn(1), outp(0), 2, 1)
    run_all(outp(1))


def _build():
    global _COMPILED
    if _COMPILED is not None:
        return _COMPILED
    import concourse.tile as tile
    from concourse import bacc, mybir

    nc = bacc.Bacc("TRN2", target_bir_lowering=False, debug=False,
                   enable_asserts=False)
    bf16 = mybir.dt.bfloat16
    f32 = mybir.dt.float32
    aps = {
        "x1": nc.dram_tensor("x1", [B, DM, N], bf16, kind="ExternalInput").ap(),
        "x2": nc.dram_tensor("x2", [B, DM, M], bf16, kind="ExternalInput").ap(),
        "wkv": nc.dram_tensor("wkv", [128, 8, 2 * LOC], bf16,
                              kind="ExternalInput").ap(),
        "wq": nc.dram_tensor("wq", [128, 8, LOC], bf16,
                             kind="ExternalInput").ap(),
        "wp": nc.dram_tensor("wp", [LOC, DM], bf16, kind="ExternalInput").ap(),
        "gk": nc.dram_tensor("gk", [128, 1], f32, kind="ExternalInput").ap(),
        "bk": nc.dram_tensor("bk", [128, 1], f32, kind="ExternalInput").ap(),
        "gq": nc.dram_tensor("gq", [128, 1], f32, kind="ExternalInput").ap(),
        "bq": nc.dram_tensor("bq", [128, 1], f32, kind="ExternalInput").ap(),
        "out": nc.dram_tensor("out", [B, N, DM], mybir.dt.float16, kind="ExternalOutput").ap(),
    }
    with tile.TileContext(nc) as tc, ExitStack() as ctx:
        _emit(ctx, tc, aps)
    nc.compile()
    _COMPILED = nc
    return nc


def kernel(x1, x2, Wq, Wk, Wv, Wp, bp, ln_g, ln_b):
    global LAST_RESULT
    from concourse.bass_utils import run_bass_kernel_spmd

    nc = _build()
    bf = ml_dtypes.bfloat16
    x1b = np.ascontiguousarray(
        np.asarray(x1, dtype=np.float32).transpose(0, 2, 1)).astype(bf)
    x2b = np.ascontiguousarray(
        np.asarray(x2, dtype=np.float32).transpose(0, 2, 1)).astype(bf)
    Wq = np.asarray(Wq, dtype=np.float32)
    Wk = np.asarray(Wk, dtype=np.float32)
    Wv = np.asarray(Wv, dtype=np.float32)
    Wp = np.asarray(Wp, dtype=np.float32)
    g = np.asarray(ln_g, dtype=np.float32)
    b_ = np.asarray(ln_b, dtype=np.float32)

    def center_heads(wT):
        w3 = wT.reshape(DM, HL, HD)
        return (w3 - w3.mean(axis=2, keepdims=True)).reshape(DM, HL * HD)

    def pack(wT, nout):
        return np.ascontiguousarray(
            wT.reshape(8, 128, nout).transpose(1, 0, 2)).astype(bf)

    g_rep = np.tile(g, HL).reshape(128, 1).astype(np.float32)
    b_rep = np.tile(b_, HL).reshape(128, 1).astype(np.float32)

    in_maps = []
    for c in range(NCORES):
        hs = slice(c * LOC, (c + 1) * LOC)
        wkT = center_heads(np.ascontiguousarray(Wk[hs, :].T))
        wvT = np.ascontiguousarray(Wv[hs, :].T)
        wqT = center_heads(np.ascontiguousarray(Wq[hs, :].T))
        wkv = np.concatenate([pack(wkT, LOC), pack(wvT, LOC)], axis=2)
        in_maps.append({
            "x1": x1b,
            "x2": x2b,
            "wkv": np.ascontiguousarray(wkv),
            "wq": pack(wqT, LOC),
            "wp": np.ascontiguousarray(Wp[:, hs].T).astype(bf),
            "gk": g_rep,
            "bk": b_rep,
            "gq": g_rep * SCALE,
            "bq": b_rep * SCALE,
        })

    res = run_bass_kernel_spmd(nc, in_maps, core_ids=list(range(NCORES)))
    LAST_RESULT = res
    acc = np.zeros((B, N, DM), dtype=np.float32)
    for r in res.results:
        acc += np.asarray(r["out"], dtype=np.float32)
    acc += np.asarray(bp, dtype=np.float32)
    return acc
